# revision 13
# baseline (speedup 1.0000x reference)
"""Trainium2 Bass kernel for nn_BTRLoss: grayscale morphological opening loss.

Per image: tip = MLP(grid, t) [16x16]; eroded = erosion(image, tip);
recon = dilation(eroded, tip); loss = mean((recon-image)^2) + regularizers.
One image per NeuronCore (data-parallel over the batch of 8).

Algorithm: the two 256-tap max-plus convolutions are computed in the
log/tropical-softmax domain so they become ordinary LINEAR 2D convolutions
that run on the (otherwise idle) 128x128 PE array instead of 512 serial
DVE min/max passes:

    eroded = -max_{u,v}(T - P)  ~=  -(1/b) ln( corr2d(exp(-b P), exp(b T)) )
    recon  =  max_{u,v}(T + E)  ~=   (1/b) ln( corr2d(exp(b E),  exp(b T)) )

with exp(b*eroded) = 1/S available as an exact elementwise reciprocal of the
erosion conv result S (no exp/log needed between the two convs).  b ~ 15 is
chosen per image so every fp32 exponent stays in range; the smooth-max bias
is O(ln(multiplicity)/b) per pixel and measured at ~4e-4 relative error on
the total loss (tolerance 2e-2); host-side prototype proto.py validates.

Device implementation per core:
- layout: rows chunked 10x113 (plus 15 halo rows = 128 partitions per
  chunk); corr2d = 16 PSUM-accumulated bf16 matmuls per [113,512] output
  tile with banded-Toeplitz stationary weights W_v[pin,pout] = K[pin-pout,v]
  (K = exp(b*tip), built on host).  2 convs x 10 chunks x 2 col-halves x 16
  taps = 640 matmuls of 512 moving rows ~ 140us PE.
- erosion tail: DVE reciprocal_approx_fast + bf16 cast; halo rebuild via 3
  band DMAs per chunk into a memset-to-1.0 padded buffer (exp(0)=1 borders
  reproduce the reference's zero padding).
- dilation tail: ACT Ln, DVE subtract of b*I (fp16 upload), ACT Square with
  per-partition accumulate; host sums 128 partials, divides by b^2, adds the
  closed-form regularizer terms (exact, from the host-computed tip MLP).
"""
import numpy as np

try:
    import concourse.bass as bass
except ImportError:
    import sys
    for p in ("/opt/trn_rl_repo", "/root/.axon_site/_ro/trn_rl_repo"):
        if p not in sys.path:
            sys.path.insert(0, p)
    import concourse.bass as bass

import ml_dtypes
import concourse.bacc as bacc
import concourse.tile as tile
from concourse import mybir
from concourse.bass_utils import run_bass_kernel_spmd

# ---- problem geometry (hardcoded per spec) ----
B, H, W = 8, 1024, 1024
K = 16
PB = 7                   # (K-1)//2 pad before
CH = 113                 # output rows per chunk (128 - 15 halo)
NCH = 10                 # ceil(1024/113)
XW = 1040                # padded-column buffer width (needs 1039)
HB = 512                 # column half width (PSUM bank = 512 fp32)

F32 = mybir.dt.float32
F16 = mybir.dt.float16
BF16 = mybir.dt.bfloat16

# tip grid (matches reference)
_x = np.linspace(-K / 2, K / 2, K, dtype=np.float32)
_X, _Y = np.meshgrid(_x, _x, indexing="ij")
XF = _X.reshape(-1)
YF = _Y.reshape(-1)


def _tip_mlp(t, w1, b1, w2, b2, w3, b3):
    inp = np.stack([XF, YF, np.full(K * K, t, np.float32)], axis=-1)
    h = np.tanh((inp @ w1 + b1).astype(np.float32)).astype(np.float32)
    h = np.tanh((h @ w2 + b2).astype(np.float32)).astype(np.float32)
    return ((h @ w3 + b3)[..., 0]).astype(np.float32)  # [256]


def build_nc():
    nc = bacc.Bacc("TRN2", target_bir_lowering=False)
    xe_d = nc.dram_tensor("xe", [128, NCH * XW], BF16, kind="ExternalInput")
    io_d = nc.dram_tensor("iout", [128, NCH * 1024], F16, kind="ExternalInput")
    w_d = nc.dram_tensor("wmat", [128, K * 128], BF16, kind="ExternalInput")
    out_d = nc.dram_tensor("psum", [128, 4 * NCH], F32, kind="ExternalOutput")

    LN = mybir.ActivationFunctionType.Ln
    SQ = mybir.ActivationFunctionType.Square
    sub = mybir.AluOpType.subtract

    with tile.TileContext(nc) as tc:
        with tc.tile_pool(name="sb", bufs=1) as sb, \
             tc.tile_pool(name="pp", bufs=4, space="PSUM") as pp, \
             tc.tile_pool(name="sc", bufs=2) as scp, \
             tc.tile_pool(name="ln", bufs=2) as lnp, \
             tc.tile_pool(name="df", bufs=2) as dfp:
            WT = sb.tile([128, K, 128], BF16)
            XeT = sb.tile([128, NCH, XW], BF16)
            YiT = sb.tile([128, NCH, 1024], BF16)   # eroded exp (interior)
            YeL = sb.tile([128, NCH, XW], BF16)     # dilation input w/ halos
            IoT = sb.tile([128, NCH, 1024], F16)    # beta * image
            ps = sb.tile([128, 4 * NCH], F32)       # [SumL2 cols | SumLI cols]

            nc.vector.memset(ps, 0.0)
            nc.vector.memset(YeL, 1.0)              # exp(0): zero-pad borders

            # --- input DMAs; W + first Xe chunks gate the matmul start, so
            # W is split and the first two Xe chunks come in column halves ---
            nc.sync.dma_start(out=WT[:, 0:K // 2, :],
                              in_=w_d[:, 0:(K // 2) * 128])
            nc.sync.dma_start(out=WT[:, K // 2:K, :],
                              in_=w_d[:, (K // 2) * 128:K * 128])
            qs = (nc.gpsimd, nc.scalar, nc.sync)
            for c in range(2):
                nc.gpsimd.dma_start(out=XeT[:, c, 0:528],
                                    in_=xe_d[:, c * XW:c * XW + 528])
                nc.scalar.dma_start(out=XeT[:, c, 528:XW],
                                    in_=xe_d[:, c * XW + 528:(c + 1) * XW])
            for c in range(2, NCH):
                qs[c % 2].dma_start(out=XeT[:, c, :],
                                    in_=xe_d[:, c * XW:(c + 1) * XW])
            for c in range(NCH):
                qs[c % 3].dma_start(out=IoT[:, c, :],
                                    in_=io_d[:, c * 1024:(c + 1) * 1024])

            # --- erosion: S = corr2d(Xe, K); Yi = bf16(1/S) ---
            for c in range(NCH):
                nv = min(CH, H - CH * c)
                for h in range(2):
                    pt = pp.tile([128, HB], F32, name="pe")
                    for v in range(K):
                        o = HB * h + v
                        nc.tensor.matmul(pt, WT[:, v, :], XeT[:, c, o:o + HB],
                                         start=(v == 0), stop=(v == K - 1))
                    rc = scp.tile([128, HB], F32, name="rc")
                    nc.vector.reciprocal_approx_fast(rc[0:CH, :], pt[0:CH, :])
                    nc.vector.tensor_scalar_add(
                        YiT[0:CH, c, HB * h:HB * (h + 1)], rc[0:CH, :], 0.0)
                # halo band DMAs into YeL (partition-shifted; DMA only)
                dq = (nc.gpsimd, nc.sync)[c % 2]
                dq.dma_start(out=YeL[PB:PB + nv, c, PB:PB + 1024],
                             in_=YiT[0:nv, c, :])
                if c + 1 < NCH:
                    dq.dma_start(out=YeL[0:PB, c + 1, PB:PB + 1024],
                                 in_=YiT[CH - PB:CH, c, :])
                if c >= 1:
                    nb = min(K - PB - 1, H - CH * c)
                    dq.dma_start(out=YeL[CH + PB:CH + PB + nb, c - 1,
                                         PB:PB + 1024],
                                 in_=YiT[0:nb, c, :])

            # --- dilation: S2 = corr2d(YeL, K); loss partials.
            # sum((L - bI)^2) = sum(L^2) - 2*sum(L*bI) + sum(bI^2): the last
            # term is exact on the host, so ACT only runs Ln+Square and DVE
            # accumulates L*bI independently -- no cross-engine chain. ---
            mul, add = mybir.AluOpType.mult, mybir.AluOpType.add
            for c in range(NCH):
                nv = min(CH, H - CH * c)
                for h in range(2):
                    pt2 = pp.tile([128, HB], F32, name="pd")
                    for v in range(K):
                        o = HB * h + v
                        nc.tensor.matmul(pt2, WT[:, v, :], YeL[:, c, o:o + HB],
                                         start=(v == 0), stop=(v == K - 1))
                    lnT = lnp.tile([128, HB], F32, name="ln")
                    nc.scalar.activation(lnT[0:nv, :], pt2[0:nv, :], LN)
                    col = 2 * c + h
                    sqT = dfp.tile([128, HB], F32, name="sq")
                    nc.scalar.activation(sqT[0:nv, :], lnT[0:nv, :], SQ,
                                         accum_out=ps[0:nv, col:col + 1])
                    liT = dfp.tile([128, HB], F32, name="li")
                    nc.vector.scalar_tensor_tensor(
                        out=liT[0:nv, :], in0=lnT[0:nv, :], scalar=1.0,
                        in1=IoT[0:nv, c, HB * h:HB * (h + 1)],
                        op0=mul, op1=mul,
                        accum_out=ps[0:nv, 2 * NCH + col:2 * NCH + col + 1])

            nc.sync.dma_start(out=out_d[:, :], in_=ps)
    nc.compile()
    return nc


_NC_CACHE = {}


def _get_nc():
    if "nc" not in _NC_CACHE:
        _NC_CACHE["nc"] = build_nc()
    return _NC_CACHE["nc"]


def _choose_beta(img, bh):
    t_max = float(bh.max())
    p_min = float(img.min())
    p_max = float(img.max())
    caps = [15.0]
    if t_max - p_min > 0:
        caps.append(79.0 / (t_max - p_min))   # erosion conv overflow
    if -p_min > 0:
        caps.append(82.0 / (-p_min))          # dilation conv underflow
    if p_max > 0:
        caps.append(79.0 / p_max)             # dilation conv overflow
    return min(caps)


def _prep_image(img, bh, beta):
    """Build the three per-core upload tensors for one image."""
    T = bh.reshape(K, K)
    Khat = np.exp(beta * T).astype(np.float32)            # [16,16]

    # banded-Toeplitz weights W[p, v, q] = Khat[p-q, v] (0 <= p-q < 16)
    p = np.arange(128)[:, None]
    q = np.arange(128)[None, :]
    d = p - q
    mask = (d >= 0) & (d < K)
    Wf = np.zeros((128, 128, K), np.float32)
    Wf[mask] = Khat[d[mask], :]
    wmat = np.ascontiguousarray(
        Wf.transpose(0, 2, 1)).reshape(128, K * 128).astype(ml_dtypes.bfloat16)

    # padded exp image, chunked with 15-row overlap: [128, 10, 1040]
    full = np.zeros((CH * (NCH - 1) + 128, XW), np.float32)
    full[PB:PB + H, PB:PB + W] = img
    Xf = np.exp(-beta * full)
    idx = (CH * np.arange(NCH))[:, None] + np.arange(128)[None, :]
    xe = np.ascontiguousarray(
        Xf[idx].transpose(1, 0, 2)).reshape(128, NCH * XW).astype(
            ml_dtypes.bfloat16)

    # beta*image in output-chunk layout: [128, 10, 1024] fp16
    rows = np.zeros((CH * (NCH - 1) + 128, W), np.float32)
    rows[0:H] = beta * img
    iout = np.ascontiguousarray(
        rows[idx].transpose(1, 0, 2)).reshape(128, NCH * 1024).astype(
            np.float16)
    # sum of (fp16-quantized beta*I)^2 -- exactly what the device multiplies
    # (over the 1024 unique image rows, not the duplicated chunk-halo rows)
    sum_i2 = float(((beta * img).astype(np.float16).astype(np.float64) ** 2)
                   .sum())
    return {"xe": xe, "iout": iout, "wmat": wmat}, sum_i2


def _prep_inputs(images, w1, b1, w2, b2, w3, b3, n):
    metas, in_maps = [], []
    for b in range(B):
        t = float(n * B + b)
        bh = _tip_mlp(t, w1, b1, w2, b2, w3, b3)
        img = images[b]
        beta = _choose_beta(img, bh)
        im, sum_i2 = _prep_image(img, bh, beta)
        metas.append((bh, beta, sum_i2))
        in_maps.append(im)
    return metas, in_maps


def _finish_loss(metas, results):
    losses = []
    for b in range(B):
        bh, beta, sum_i2 = metas[b]
        p = np.asarray(results[b]["psum"], np.float64)
        sum_l2 = float(p[:, 0:2 * NCH].sum())
        sum_li = float(p[:, 2 * NCH:4 * NCH].sum())
        s = sum_l2 - 2.0 * sum_li + sum_i2
        recon = s / (beta * beta) / (H * W)
        tip = bh.reshape(K, K)
        boundary = float(np.mean((bh + 100.0) ** 2))
        reg = float(np.sum(bh ** 2))
        cent = float(np.dot(np.abs(bh), XF)) ** 2 + \
            float(np.dot(np.abs(bh), YF)) ** 2
        avg = float(np.mean(bh)) ** 2
        height = float(np.mean(np.maximum(tip, 0.0) ** 2)) + \
            float(np.max(tip)) ** 2
        losses.append(recon + 0.1 * boundary + 1.0 * height
                      + 1e-4 * reg + 0.1 * avg + 1e-3 * cent)
    return np.array(np.mean(np.asarray(losses, np.float64)), dtype=np.float32)


def _run(inputs, trace=False, **kw):
    images = np.asarray(inputs["images"], np.float32)
    args = [np.asarray(inputs[k], np.float32)
            for k in ("w1", "b1", "w2", "b2", "w3", "b3")]
    n = int(np.asarray(inputs["n"]))
    metas, in_maps = _prep_inputs(images, *args, n)
    res = run_bass_kernel_spmd(_get_nc(), in_maps, core_ids=list(range(B)),
                               trace=trace, **kw)
    return _finish_loss(metas, res.results), res


def kernel(**inputs) -> np.ndarray:
    loss, _ = _run(inputs)
    return loss
